# revision 2
# baseline (speedup 1.0000x reference)
"""AttentionMemoryInterface Trainium2 kernel.

Reference computation per batch element b (memory [N=4096, D=128], x [256]):
    mv = x@W_write+b_write; wq = x@W_wq+b_wq; rq = x@W_rq+b_rq
    wl[n] = mem[n,:]@wq ; ww = softmax(wl)
    new_mem = mem*(1-ww) + mv*ww
    rl[n] = new_mem[n,:]@rq ; rw = softmax(rl)
    out = (rw @ new_mem) @ W_ro + b_ro

Algebraic restructure (new_mem never materialized):
    lr[n] = mem[n,:]@rq                  (computed in the same pass as wl)
    cbar  = mv@rq                        (scalar per b)
    rl[n] = lr[n] + ww[n]*(cbar - lr[n])
    g[n]  = rw[n]*(1-ww[n]);  s = sum_n rw[n]*ww[n]
    read_out = sum_n g[n]*mem[n,:] + s*mv
    out = read_out @ W_ro + b_ro

So memory is streamed through the PE exactly twice:
  pass 1: per 128-slot chunk, PE-transpose (natural [n,d] -> [d,n]) then
          matmul vs all queries -> logits
  pass 2: natural chunk as stationary, g-column as moving -> PSUM-accumulated
          read_out

Sharding: data-parallel over batch (8 per core), weights replicated.
"""

import numpy as np

import concourse.bass as bass
import concourse.bacc as bacc
import concourse.mybir as mybir
import concourse.tile as tile
from concourse.bass_utils import run_bass_kernel_spmd

N_CORES = 8
B, IN_DIM, D, N_SLOTS = 64, 256, 128, 4096
BC = B // N_CORES          # batch per core
NCH = N_SLOTS // 128       # n-chunks per batch element
F32 = mybir.dt.float32
AX = mybir.AxisListType
ALU = mybir.AluOpType
ACTF = mybir.ActivationFunctionType


def build_nc():
    nc = bacc.Bacc("TRN2", target_bir_lowering=False, debug=False,
                   num_devices=N_CORES)

    x_d = nc.dram_tensor("x", [BC, IN_DIM], F32, kind="ExternalInput")
    mem_d = nc.dram_tensor("memory", [BC, N_SLOTS, D], F32, kind="ExternalInput")
    w_wr_d = nc.dram_tensor("W_write", [IN_DIM, D], F32, kind="ExternalInput")
    b_wr_d = nc.dram_tensor("b_write", [1, D], F32, kind="ExternalInput")
    w_wq_d = nc.dram_tensor("W_wq", [IN_DIM, D], F32, kind="ExternalInput")
    b_wq_d = nc.dram_tensor("b_wq", [1, D], F32, kind="ExternalInput")
    w_rq_d = nc.dram_tensor("W_rq", [IN_DIM, D], F32, kind="ExternalInput")
    b_rq_d = nc.dram_tensor("b_rq", [1, D], F32, kind="ExternalInput")
    w_ro_d = nc.dram_tensor("W_ro", [D, IN_DIM], F32, kind="ExternalInput")
    b_ro_d = nc.dram_tensor("b_ro", [1, IN_DIM], F32, kind="ExternalInput")
    ident_d = nc.dram_tensor("ident", [128, 128], F32, kind="ExternalInput")
    onesc_d = nc.dram_tensor("ones_col", [128, 1], F32, kind="ExternalInput")
    onesr_d = nc.dram_tensor("ones_row", [1, 128], F32, kind="ExternalInput")
    out_d = nc.dram_tensor("out", [BC, IN_DIM], F32, kind="ExternalOutput")

    with tile.TileContext(nc) as tc:
        _body(nc, tc,
              x=x_d.ap(), mem=mem_d.ap(),
              w_wr=w_wr_d.ap(), b_wr=b_wr_d.ap(),
              w_wq=w_wq_d.ap(), b_wq=b_wq_d.ap(),
              w_rq=w_rq_d.ap(), b_rq=b_rq_d.ap(),
              w_ro=w_ro_d.ap(), b_ro=b_ro_d.ap(),
              ident=ident_d.ap(), ones_col=onesc_d.ap(),
              ones_row=onesr_d.ap(), out=out_d.ap())
    nc.compile()
    return nc


def _body(nc, tc, *, x, mem, w_wr, b_wr, w_wq, b_wq, w_rq, b_rq,
          w_ro, b_ro, ident, ones_col, ones_row, out):
    from contextlib import ExitStack
    ctx = ExitStack()
    with ctx:
        consts = ctx.enter_context(tc.tile_pool(name="consts", bufs=1))
        mempool = ctx.enter_context(tc.tile_pool(name="mem", bufs=1))
        work = ctx.enter_context(tc.tile_pool(name="work", bufs=3))
        sm = ctx.enter_context(tc.tile_pool(name="sm", bufs=2))
        ps_t = ctx.enter_context(tc.tile_pool(name="ps_t", bufs=2, space="PSUM"))
        ps_l = ctx.enter_context(tc.tile_pool(name="ps_l", bufs=2, space="PSUM"))
        ps_acc = ctx.enter_context(tc.tile_pool(name="ps_acc", bufs=1, space="PSUM"))
        ps_sm = ctx.enter_context(tc.tile_pool(name="ps_sm", bufs=2, space="PSUM"))

        # ---------- constants ----------
        ident_sb = consts.tile([128, 128], F32, tag="ident")
        nc.sync.dma_start(ident_sb[:], ident)
        onesc_sb = consts.tile([128, 1], F32, tag="onesc")
        nc.sync.dma_start(onesc_sb[:], ones_col)
        onesr_sb = consts.tile([1, 128], F32, tag="onesr")
        nc.sync.dma_start(onesr_sb[:], ones_row)

        w_ro_sb = consts.tile([D, IN_DIM], F32, tag="wro")
        nc.sync.dma_start(w_ro_sb[:], w_ro)
        b_ro_sb = consts.tile([1, IN_DIM], F32, tag="bro")
        nc.sync.dma_start(b_ro_sb[:], b_ro)

        proj_w = []
        for name, wd, bd in (("wr", w_wr, b_wr), ("wq", w_wq, b_wq),
                             ("rq", w_rq, b_rq)):
            chunks = []
            for k in range(IN_DIM // 128):
                wt = consts.tile([128, D], F32, tag=f"w_{name}{k}")
                nc.sync.dma_start(wt[:], wd[k * 128:(k + 1) * 128, :])
                chunks.append(wt)
            bt = consts.tile([1, D], F32, tag=f"b_{name}")
            nc.sync.dma_start(bt[:], bd)
            proj_w.append((chunks, bt))

        x_nat = consts.tile([BC, IN_DIM], F32, tag="xnat")
        nc.sync.dma_start(x_nat[:], x)

        # ---------- x transpose: X_T[k] = x[:, 128k:128k+128].T  [128, BC] ----------
        xt = []
        for k in range(IN_DIM // 128):
            ps = ps_sm.tile([128, BC], F32, tag="ps_small")
            nc.tensor.matmul(ps[:], x_nat[:, k * 128:(k + 1) * 128],
                             ident_sb[0:BC, 0:BC], is_transpose=True)
            t = consts.tile([128, BC], F32, tag=f"xt{k}")
            nc.scalar.activation(t[:], ps[:], ACTF.Copy)
            xt.append(t)

        # ---------- projections (transposed): proj_T[j] = (x @ W + b).T  [128, BC] ----------
        # mv_T, and QP_ALL = [wq_T | rq_T]  [128, 2*BC]
        mv_t = consts.tile([128, BC], F32, tag="mvt")
        qp_all = consts.tile([128, 2 * BC], F32, tag="qpall")
        for j, (chunks, bt) in enumerate(proj_w):
            ps = ps_sm.tile([128, BC], F32, tag="ps_small")
            # bias broadcast along batch: out[j, b] = bias[j]
            nc.tensor.matmul(ps[:], bt[:], onesr_sb[:, 0:BC], start=True,
                             stop=False)
            for k in range(IN_DIM // 128):
                nc.tensor.matmul(ps[:], chunks[k][:], xt[k][:],
                                 start=False, stop=(k == IN_DIM // 128 - 1))
            if j == 0:
                nc.scalar.activation(mv_t[:], ps[:], ACTF.Copy)
            elif j == 1:
                nc.scalar.activation(qp_all[:, 0:BC], ps[:], ACTF.Copy)
            else:
                nc.scalar.activation(qp_all[:, BC:2 * BC], ps[:], ACTF.Copy)

        # ---------- cbar[b] = mv[b] . rq[b] ----------
        tmv = sm.tile([128, BC], F32, tag="tmv")
        nc.vector.tensor_tensor(tmv[:], mv_t[:], qp_all[:, BC:2 * BC], ALU.mult)
        ps_c = ps_sm.tile([1, BC], F32, tag="ps_small")
        nc.tensor.matmul(ps_c[:], onesc_sb[:], tmv[:])
        c_row = consts.tile([1, BC], F32, tag="crow")
        nc.scalar.activation(c_row[:], ps_c[:], ACTF.Copy)
        ps_cb = ps_sm.tile([128, BC], F32, tag="ps_small")
        nc.tensor.matmul(ps_cb[:], onesr_sb[:], c_row[:])
        c_bc = consts.tile([128, BC], F32, tag="cbc")
        nc.scalar.activation(c_bc[:], ps_cb[:], ACTF.Copy)

        # ---------- per-batch-element memory streams ----------
        m_tiles = []
        for b in range(BC):
            mb = mempool.tile([128, NCH * D], F32, tag=f"mem{b}")
            nc.sync.dma_start(
                mb[:].rearrange("p (c d) -> p c d", d=D),
                mem[b].rearrange("(c p) d -> p c d", p=128))
            m_tiles.append(mb)

        # accumulators shared across b
        ps_ro = ps_acc.tile([128, BC], F32, tag="ps_ro")
        ps_srow = ps_acc.tile([1, BC], F32, tag="ps_srow")

        g_tiles = []
        for b in range(BC):
            mb = m_tiles[b]
            # ----- pass 1: logits -----
            ps_lb = ps_l.tile([128, NCH * 2 * BC], F32, tag="ps_lb")
            for grp in range(NCH // 4):
                ps_tt = ps_t.tile([128, 512], F32, tag="ps_tt")
                for j in range(4):
                    c = 4 * grp + j
                    nc.tensor.matmul(ps_tt[:, j * 128:(j + 1) * 128],
                                     mb[:, c * 128:(c + 1) * 128],
                                     ident_sb[:], is_transpose=True)
                tg = work.tile([128, 512], F32, tag="tgrp")
                nc.any.tensor_copy(tg[:], ps_tt[:])
                for j in range(4):
                    c = 4 * grp + j
                    nc.tensor.matmul(
                        ps_lb[:, c * 16:(c + 1) * 16],
                        tg[:, j * 128:(j + 1) * 128], qp_all[:],
                        start=True, stop=True)

            # extract wl (k=0) / lr (k=1) -> wlr [128, 2, 32]; wl=[:,0,:], lr=[:,1,:]
            wlr = sm.tile([128, 2 * NCH], F32, tag="wlr")
            nc.scalar.activation(
                wlr[:].rearrange("p (k c) -> p c k", k=2),
                ps_lb[:, b::BC].rearrange("p (c k) -> p c k", k=2),
                ACTF.Copy)
            wl = wlr[:, 0:NCH]
            lr = wlr[:, NCH:2 * NCH]

            # ----- softmax 1 (no max-subtraction; |logits| < 60 safe in fp32) -----
            e1 = sm.tile([128, NCH], F32, tag="e1")
            e1s = sm.tile([128, 1], F32, tag="e1s")
            nc.scalar.activation(e1[:], wl, ACTF.Exp, accum_out=e1s[:])
            ps_s1 = ps_sm.tile([1, 1], F32, tag="ps_small")
            nc.tensor.matmul(ps_s1[:], e1s[:], onesc_sb[:, 0:1])
            s1 = sm.tile([1, 1], F32, tag="s1")
            nc.any.tensor_copy(s1[:], ps_s1[:])
            r1 = sm.tile([1, 1], F32, tag="r1")
            nc.vector.reciprocal(r1[:], s1[:])
            ps_r1 = ps_sm.tile([128, 1], F32, tag="ps_small")
            nc.tensor.matmul(ps_r1[:], onesr_sb[:], r1[:])
            r1c = sm.tile([128, 1], F32, tag="r1c")
            nc.any.tensor_copy(r1c[:], ps_r1[:])
            ww = sm.tile([128, NCH], F32, tag="ww")
            nc.vector.tensor_scalar_mul(ww[:], e1[:], r1c[:])

            # ----- read logits: rl = lr + ww*(cbar - lr) -----
            t1 = sm.tile([128, NCH], F32, tag="t1")
            nc.vector.scalar_tensor_tensor(t1[:], lr, c_bc[:, b:b + 1], ww[:],
                                           op0=ALU.subtract, op1=ALU.mult)
            # t1 = (lr - cbar) * ww ; rl = lr - t1
            rl = sm.tile([128, NCH], F32, tag="rl")
            nc.vector.tensor_tensor(rl[:], lr, t1[:], ALU.subtract)

            # ----- softmax 2 (unnormalized) + g -----
            e2 = sm.tile([128, NCH], F32, tag="e2")
            e2s = sm.tile([128, 1], F32, tag="e2s")
            nc.scalar.activation(e2[:], rl[:], ACTF.Exp, accum_out=e2s[:])
            ps_s2 = ps_sm.tile([1, 1], F32, tag="ps_small")
            nc.tensor.matmul(ps_s2[:], e2s[:], onesc_sb[:, 0:1])
            s2 = sm.tile([1, 1], F32, tag="s2")
            nc.any.tensor_copy(s2[:], ps_s2[:])
            r2 = sm.tile([1, 1], F32, tag="r2")
            nc.vector.reciprocal(r2[:], s2[:])
            ps_r2 = ps_sm.tile([128, 1], F32, tag="ps_small")
            nc.tensor.matmul(ps_r2[:], onesr_sb[:], r2[:])
            r2c = sm.tile([128, 1], F32, tag="r2c")
            nc.any.tensor_copy(r2c[:], ps_r2[:])
            rw = sm.tile([128, NCH], F32, tag="rw")
            nc.vector.tensor_scalar_mul(rw[:], e2[:], r2c[:])

            t2 = sm.tile([128, NCH], F32, tag="t2")
            nc.vector.tensor_tensor(t2[:], rw[:], ww[:], ALU.mult)
            g = sm.tile([128, NCH], F32, tag=f"g{b}")
            nc.vector.tensor_tensor(g[:], rw[:], t2[:], ALU.subtract)
            g_tiles.append(g)
            # s_b = sum(rw*ww) -> ps_srow[0, b]
            t2s = sm.tile([128, 1], F32, tag="t2s")
            nc.vector.tensor_reduce(t2s[:], t2[:], AX.X, ALU.add)
            nc.tensor.matmul(ps_srow[0:1, b:b + 1], t2s[:], onesc_sb[:, 0:1])

            # ----- pass 2: read_out[:, b] = sum_c mem_chunk_c.T @ g[:, c] -----
            for c in range(NCH):
                nc.tensor.matmul(ps_ro[:, b:b + 1],
                                 mb[:, c * 128:(c + 1) * 128],
                                 g[:, c:c + 1],
                                 start=(c == 0), stop=(c == NCH - 1))

        # ---------- epilogue ----------
        ro_t = sm.tile([128, BC], F32, tag="rot")
        nc.any.tensor_copy(ro_t[:], ps_ro[:])
        s_row = sm.tile([1, BC], F32, tag="srow")
        nc.any.tensor_copy(s_row[:], ps_srow[:])
        ps_sbc = ps_sm.tile([128, BC], F32, tag="ps_small")
        nc.tensor.matmul(ps_sbc[:], onesr_sb[:], s_row[:])
        s_bc = sm.tile([128, BC], F32, tag="sbc")
        nc.any.tensor_copy(s_bc[:], ps_sbc[:])

        # ro2 = ro + s*mv   [128, BC]
        t3 = sm.tile([128, BC], F32, tag="t3")
        nc.vector.tensor_tensor(t3[:], mv_t[:], s_bc[:], ALU.mult)
        ro2 = sm.tile([128, BC], F32, tag="ro2")
        nc.vector.tensor_tensor(ro2[:], ro_t[:], t3[:], ALU.add)

        # out = ro2.T @ W_ro + b_ro   [BC, IN_DIM]
        ps_out = ps_sm.tile([BC, IN_DIM], F32, tag="ps_small")
        nc.tensor.matmul(ps_out[:], onesr_sb[:, 0:BC], b_ro_sb[:],
                         start=True, stop=False)
        nc.tensor.matmul(ps_out[:], ro2[:], w_ro_sb[:], start=False, stop=True)
        out_sb = sm.tile([BC, IN_DIM], F32, tag="outsb")
        nc.any.tensor_copy(out_sb[:], ps_out[:])
        nc.sync.dma_start(out, out_sb[:])


_NC_CACHE = None


def _get_nc():
    global _NC_CACHE
    if _NC_CACHE is None:
        _NC_CACHE = build_nc()
    return _NC_CACHE


def make_in_maps(inputs):
    ident = np.eye(128, dtype=np.float32)
    ones_col = np.ones((128, 1), dtype=np.float32)
    ones_row = np.ones((1, 128), dtype=np.float32)
    shared = {
        "W_write": np.ascontiguousarray(inputs["W_write"], dtype=np.float32),
        "b_write": np.ascontiguousarray(inputs["b_write"], dtype=np.float32).reshape(1, D),
        "W_wq": np.ascontiguousarray(inputs["W_wq"], dtype=np.float32),
        "b_wq": np.ascontiguousarray(inputs["b_wq"], dtype=np.float32).reshape(1, D),
        "W_rq": np.ascontiguousarray(inputs["W_rq"], dtype=np.float32),
        "b_rq": np.ascontiguousarray(inputs["b_rq"], dtype=np.float32).reshape(1, D),
        "W_ro": np.ascontiguousarray(inputs["W_ro"], dtype=np.float32),
        "b_ro": np.ascontiguousarray(inputs["b_ro"], dtype=np.float32).reshape(1, IN_DIM),
        "ident": ident, "ones_col": ones_col, "ones_row": ones_row,
    }
    x = np.ascontiguousarray(inputs["x"], dtype=np.float32)
    mem = np.ascontiguousarray(inputs["memory"], dtype=np.float32)
    in_maps = []
    for i in range(N_CORES):
        m = dict(shared)
        m["x"] = np.ascontiguousarray(x[i * BC:(i + 1) * BC])
        m["memory"] = np.ascontiguousarray(mem[i * BC:(i + 1) * BC])
        in_maps.append(m)
    return in_maps


def kernel(**inputs) -> np.ndarray:
    nc = _get_nc()
    in_maps = make_in_maps(inputs)
    res = run_bass_kernel_spmd(nc, in_maps, list(range(N_CORES)))
    out = np.concatenate([res.results[i]["out"] for i in range(N_CORES)], axis=0)
    return np.ascontiguousarray(out, dtype=np.float32)


if __name__ == "__main__":
    nc = build_nc()
    print("built ok; instructions:",
          sum(len(bb.instructions) for bb in nc.main_func.blocks))
